# revision 1
# baseline (speedup 1.0000x reference)
import sys

sys.path.insert(0, "/opt/trn_rl_repo")

import numpy as np

N = 4096
B = 8192
N_CORES = 8
B_SHARD = B // N_CORES
NB = B_SHARD // 128

_STATE = {}


def _build():
    import concourse.bacc as bacc
    import concourse.mybir as mybir
    import concourse.tile as tile
    import bass_rust

    f32 = mybir.dt.float32
    f32r = mybir.dt.float32r
    ADD = mybir.AluOpType.add
    SUB = mybir.AluOpType.subtract
    mmdt = f32r

    nc = bacc.Bacc("TRN2", target_bir_lowering=False, debug=False)
    xtr_d = nc.declare_dram_parameter("xtr", [N, B_SHARD], f32, isOutput=False)
    w3_d = nc.declare_dram_parameter("w3", [3 * N], f32, isOutput=False)
    out_d = nc.declare_dram_parameter("out", [B_SHARD, N], f32, isOutput=True)

    xtr_t = xtr_d[:].rearrange("(a p) b -> p a b", p=128)

    with tile.TileContext(nc) as tc:
        with (
            tc.tile_pool(name="const", bufs=1) as constp,
            tc.tile_pool(name="xbig", bufs=2) as xbigp,
            tc.tile_pool(name="xplus", bufs=18) as xpp_pool,
            tc.tile_pool(name="xmm", bufs=9) as xmp,
            tc.tile_pool(name="outp", bufs=2) as op,
            tc.tile_pool(name="psum", bufs=1, space="PSUM") as pp,
        ):
            band_mh = constp.tile([128, 3968], mmdt)
            band_pmh = constp.tile([128, 1920], mmdt)
            band_3ph = constp.tile([128, 896], mmdt)
            band_3mh = constp.tile([128, 896], mmdt)

            warm_in = constp.tile([128, 512], mmdt, name="warm_in")
            warm_f = constp.tile([128, 512], f32, name="warm_f")
            nc.gpsimd.memset(warm_f[:], 0.0)
            nc.vector.tensor_copy(warm_in[:], warm_f[:])

            def emit_nega(bt, xbig):
                s_m = pp.tile([128, 2048], f32, tag="sm", name="sm")
                if bt == 0:
                    for _ in range(32):
                        nc.tensor.matmul(
                            s_m[:, 0:512],
                            warm_in[:, 0:128],
                            warm_in[:],
                            start=True,
                            stop=True,
                        )
                xplus = []
                for t in range(16):
                    xpl = xpp_pool.tile([128, 128], f32, tag="xp", name="xp")
                    nc.gpsimd.tensor_tensor(
                        xpl[:], xbig[:, 31 - t, :], xbig[:, 15 - t, :], ADD
                    )
                    xplus.append(xpl)
                    xm = xmp.tile([128, 128], mmdt, tag="xm", name="xm")
                    nc.vector.tensor_tensor(
                        xm[:], xbig[:, 31 - t, :], xbig[:, 15 - t, :], SUB
                    )
                    q0 = (N - 128) - 128 * t
                    for j in range(4):
                        u = q0 - 2048 + 512 * j
                        nc.tensor.matmul(
                            s_m[:, 512 * j : 512 * j + 512],
                            xm[:],
                            band_mh[:, u : u + 512],
                            start=(t == 0),
                            stop=(t == 15),
                        )
                return s_m, xplus

            def emit_level23(bt, xplus):
                s_pm = pp.tile([128, 1024], f32, tag="spm", name="spm")
                s_3p = pp.tile([128, 512], f32, tag="s3p", name="s3p")
                s_3m = pp.tile([128, 512], f32, tag="s3m", name="s3m")
                xpp2 = []
                for t in range(8):
                    xq = xpp_pool.tile([128, 128], f32, tag="xq", name="xq", bufs=10)
                    nc.gpsimd.tensor_tensor(
                        xq[:], xplus[t][:], xplus[t + 8][:], ADD
                    )
                    xpp2.append(xq)
                    xpm = xmp.tile([128, 128], mmdt, tag="xpm", name="xpm")
                    nc.vector.tensor_tensor(
                        xpm[:], xplus[t][:], xplus[t + 8][:], SUB
                    )
                    q0pm = (2048 - 128) - 128 * t
                    for j in range(2):
                        u = q0pm - 1024 + 512 * j
                        nc.tensor.matmul(
                            s_pm[:, 512 * j : 512 * j + 512],
                            xpm[:],
                            band_pmh[:, u : u + 512],
                            start=(t == 0),
                            stop=(t == 7),
                        )
                for t in range(4):
                    x3p = xmp.tile([128, 128], mmdt, tag="x3p", name="x3p")
                    nc.vector.tensor_tensor(
                        x3p[:], xpp2[t][:], xpp2[t + 4][:], ADD
                    )
                    x3m = xmp.tile([128, 128], mmdt, tag="x3m", name="x3m")
                    nc.vector.tensor_tensor(
                        x3m[:], xpp2[t][:], xpp2[t + 4][:], SUB
                    )
                    q03p = (512 - 128) - 128 * t
                    q03m = (1024 - 128) - 128 * t
                    nc.tensor.matmul(
                        s_3p[:],
                        x3p[:],
                        band_3ph[:, q03p : q03p + 512],
                        start=(t == 0),
                        stop=(t == 3),
                    )
                    nc.tensor.matmul(
                        s_3m[:],
                        x3m[:],
                        band_3mh[:, q03m - 512 : q03m - 512 + 512],
                        start=(t == 0),
                        stop=(t == 3),
                    )
                return s_pm, s_3p, s_3m

            def emit_copies(s_m, s_pm, s_3p, s_3m):
                cm = op.tile([128, 2048], f32, tag="cm", name="cm")
                nc.scalar.mul(cm[:], s_m[:], 0.5)
                c3p = op.tile([128, 512], f32, tag="c3p", name="c3p")
                nc.scalar.mul(c3p[:], s_3p[:], 0.125)
                c3m = op.tile([128, 512], f32, tag="c3m", name="c3m")
                nc.scalar.mul(c3m[:], s_3m[:], 0.125)
                cpm = op.tile([128, 1024], f32, tag="cpm", name="cpm")
                nc.scalar.mul(cpm[:], s_pm[:], 0.25)
                return cm, c3p, c3m, cpm

            def make_unfold(b0, cm, c3p, c3m, cpm):
                def unfold():
                    cpp = op.tile([128, 1024], f32, tag="cpp", name="cpp")
                    nc.vector.tensor_tensor(cpp[:, 0:512], c3p[:], c3m[:], ADD)
                    nc.vector.tensor_tensor(cpp[:, 512:1024], c3p[:], c3m[:], SUB)
                    u1 = op.tile([128, 1024], f32, tag="u1", name="u1")
                    nc.vector.tensor_tensor(u1[:], cpp[:], cpm[:], ADD)
                    u2 = op.tile([128, 1024], f32, tag="u2", name="u2")
                    nc.vector.tensor_tensor(u2[:], cpp[:], cpm[:], SUB)
                    for seg, (usrc, moff, alu) in enumerate(
                        ((u1, 0, ADD), (u2, 1024, ADD), (u1, 0, SUB), (u2, 1024, SUB))
                    ):
                        o = op.tile([128, 1024], f32, tag="o", name="o", bufs=4)
                        nc.vector.tensor_tensor(
                            o[:], usrc[:], cm[:, moff : moff + 1024], alu
                        )
                        nc.sync.dma_start(
                            out_d[b0 : b0 + 128, 1024 * seg : 1024 * seg + 1024],
                            o[:],
                        )

                return unfold

            with tc.tile_pool(name="scratch", bufs=1) as scr:
                W = 6016
                band_wf = scr.tile([128, W], f32)
                srcA = bass_rust.AP(
                    tensor=w3_d[:].tensor,
                    offset=1 + 1920,
                    ap=[[1, 128], [1, W - 1920]],
                )
                nc.sync.dma_start(band_wf[:, 1920:W], srcA)
                srcB = bass_rust.AP(
                    tensor=w3_d[:].tensor, offset=1, ap=[[1, 128], [1, 1920]]
                )
                nc.sync.dma_start(band_wf[:, 0:1920], srcB)

                xbig0 = xbigp.tile([128, 32, 128], f32, tag="xbig", name="xbig0")
                nc.sync.dma_start(xbig0[:, 24:32, :], xtr_t[:, 24:32, 0:128])
                nc.sync.dma_start(xbig0[:, 8:16, :], xtr_t[:, 8:16, 0:128])
                nc.sync.dma_start(xbig0[:, 16:24, :], xtr_t[:, 16:24, 0:128])
                nc.sync.dma_start(xbig0[:, 0:8, :], xtr_t[:, 0:8, 0:128])

                nc.vector.tensor_tensor(
                    band_mh[:, 1920:3968],
                    band_wf[:, 3968 : 3968 + 2048],
                    band_wf[:, 1920 : 1920 + 2048],
                    SUB,
                )
                nc.vector.tensor_tensor(
                    band_mh[:, 0:1920],
                    band_wf[:, 2048 : 2048 + 1920],
                    band_wf[:, 0:1920],
                    SUB,
                )

                s_m0, xplus0 = emit_nega(0, xbig0)

                band_p = scr.tile([128, 2048], f32)
                nc.vector.tensor_tensor(
                    band_p[:], band_wf[:, 0:2048], band_wf[:, 2048:4096], ADD
                )
                nc.vector.tensor_tensor(
                    band_pmh[:, 0:1024], band_p[:, 1024:2048], band_p[:, 0:1024], SUB
                )
                nc.vector.tensor_tensor(
                    band_pmh[:, 1024:1920],
                    band_p[:, 0:896],
                    band_p[:, 1024 : 1024 + 896],
                    SUB,
                )
                band_pp = scr.tile([128, 1024], f32)
                nc.vector.tensor_tensor(
                    band_pp[:], band_p[:, 0:1024], band_p[:, 1024:2048], ADD
                )
                nc.vector.tensor_tensor(
                    band_3ph[:, 0:512], band_pp[:, 0:512], band_pp[:, 512:1024], ADD
                )
                nc.vector.tensor_tensor(
                    band_3ph[:, 512:896], band_pp[:, 512:896], band_pp[:, 0:384], ADD
                )
                nc.vector.tensor_tensor(
                    band_3mh[:, 0:512], band_pp[:, 512:1024], band_pp[:, 0:512], SUB
                )
                nc.vector.tensor_tensor(
                    band_3mh[:, 512:896], band_pp[:, 0:384], band_pp[:, 512:896], SUB
                )

            def emit_folds_only(xbig):
                xplus = []
                for t in range(16):
                    xpl = xpp_pool.tile([128, 128], f32, tag="xp", name="xp")
                    nc.gpsimd.tensor_tensor(
                        xpl[:], xbig[:, 31 - t, :], xbig[:, 15 - t, :], ADD
                    )
                    xplus.append(xpl)
                return xplus

            def emit_nega_mms(xplus_src, xbig):
                s_m = pp.tile([128, 2048], f32, tag="sm", name="sm")
                for t in range(16):
                    xm = xmp.tile([128, 128], mmdt, tag="xm", name="xm")
                    nc.vector.tensor_tensor(
                        xm[:], xbig[:, 31 - t, :], xbig[:, 15 - t, :], SUB
                    )
                    q0 = (N - 128) - 128 * t
                    for j in range(4):
                        u = q0 - 2048 + 512 * j
                        nc.tensor.matmul(
                            s_m[:, 512 * j : 512 * j + 512],
                            xm[:],
                            band_mh[:, u : u + 512],
                            start=(t == 0),
                            stop=(t == 15),
                        )
                return s_m

            s_pm0, s_3p0, s_3m0 = emit_level23(0, xplus0)
            pending = make_unfold(0, *emit_copies(s_m0, s_pm0, s_3p0, s_3m0))

            for bt in range(1, NB - 1):
                b0 = 128 * bt
                xbig = xbigp.tile([128, 32, 128], f32, tag="xbig", name="xbig")
                nc.sync.dma_start(xbig[:], xtr_t[:, :, b0 : b0 + 128])
                s_m, xplus = emit_nega(bt, xbig)
                s_pm, s_3p, s_3m = emit_level23(bt, xplus)
                copies = emit_copies(s_m, s_pm, s_3p, s_3m)
                pending()
                pending = make_unfold(b0, *copies)

            b0 = 128 * (NB - 1)
            xbig = xbigp.tile([128, 32, 128], f32, tag="xbig", name="xbig")
            nc.sync.dma_start(xbig[:], xtr_t[:, :, b0 : b0 + 128])
            xplus = emit_folds_only(xbig)
            s_pm, s_3p, s_3m = emit_level23(NB - 1, xplus)
            c3p = op.tile([128, 512], f32, tag="c3p", name="c3p")
            nc.scalar.mul(c3p[:], s_3p[:], 0.125)
            c3m = op.tile([128, 512], f32, tag="c3m", name="c3m")
            nc.scalar.mul(c3m[:], s_3m[:], 0.125)
            cpm = op.tile([128, 1024], f32, tag="cpm", name="cpm")
            nc.scalar.mul(cpm[:], s_pm[:], 0.25)
            s_m = emit_nega_mms(xplus, xbig)
            pending()
            cpp = op.tile([128, 1024], f32, tag="cpp", name="cpp")
            nc.vector.tensor_tensor(cpp[:, 0:512], c3p[:], c3m[:], ADD)
            nc.vector.tensor_tensor(cpp[:, 512:1024], c3p[:], c3m[:], SUB)
            u1 = op.tile([128, 1024], f32, tag="u1", name="u1")
            nc.vector.tensor_tensor(u1[:], cpp[:], cpm[:], ADD)
            u2 = op.tile([128, 1024], f32, tag="u2", name="u2")
            nc.vector.tensor_tensor(u2[:], cpp[:], cpm[:], SUB)
            cm = op.tile([128, 2048], f32, tag="cm", name="cm")
            combos = {0: (u1, 0, ADD), 1: (u2, 1024, ADD), 2: (u1, 0, SUB), 3: (u2, 1024, SUB)}
            os_ = {}
            for seg in range(4):
                os_[seg] = op.tile([128, 1024], f32, tag="o", name="o", bufs=4)
            for q in range(4):
                c0 = 512 * q
                nc.scalar.mul(cm[:, c0 : c0 + 512], s_m[:, c0 : c0 + 512], 0.5)
                half = q % 2
                blk = q // 2
                for seg in (blk, blk + 2):
                    usrc, moff, alu = combos[seg]
                    h0 = moff + 512 * half
                    o = os_[seg]
                    nc.vector.tensor_tensor(
                        o[:, 512 * half : 512 * half + 512],
                        usrc[:, 512 * half : 512 * half + 512],
                        cm[:, h0 : h0 + 512],
                        alu,
                    )
                    nc.sync.dma_start(
                        out_d[
                            b0 : b0 + 128,
                            1024 * seg + 512 * half : 1024 * seg + 512 * half + 512,
                        ],
                        o[:, 512 * half : 512 * half + 512],
                    )

    nc.compile()
    return nc


def _get_nc():
    if "nc" not in _STATE:
        _STATE["nc"] = _build()
    return _STATE["nc"]


def _prep_inputs(x, w):
    x = np.ascontiguousarray(x, dtype=np.float32)
    w = np.ascontiguousarray(w, dtype=np.float32)
    wrev = np.roll(w[::-1], 1)
    w3 = np.tile(wrev, 3)
    in_maps = []
    for i in range(N_CORES):
        xtr = np.ascontiguousarray(x[i * B_SHARD : (i + 1) * B_SHARD, ::-1].T)
        in_maps.append({"xtr": xtr, "w3": w3})
    return in_maps


def kernel(x, w, _trace=False):
    from concourse.bass_utils import run_bass_kernel_spmd

    nc = _get_nc()
    in_maps = _prep_inputs(x, w)
    res = run_bass_kernel_spmd(nc, in_maps, list(range(N_CORES)), trace=_trace)
    out = np.concatenate([res.results[i]["out"] for i in range(N_CORES)], axis=0)
    if _trace:
        _STATE["last_result"] = res
    return out



# revision 15
# speedup vs baseline: 1.5390x; 1.5390x over previous
import sys

sys.path.insert(0, "/opt/trn_rl_repo")

import numpy as np
import ml_dtypes

N = 4096
B = 8192
N_CORES = 8
B_SHARD = B // N_CORES
NB = B_SHARD // 128
SQ2 = float(np.sqrt(2.0))
ISQ2 = float(1.0 / np.sqrt(2.0))

_STATE = {}


def _build():
    import concourse.bacc as bacc
    import concourse.mybir as mybir
    import concourse.tile as tile

    f32 = mybir.dt.float32
    bf16 = mybir.dt.bfloat16
    ADD = mybir.AluOpType.add
    SUB = mybir.AluOpType.subtract
    MUL = mybir.AluOpType.mult

    nc = bacc.Bacc("TRN2", target_bir_lowering=False, debug=False)
    nc._dbg_labels = {}

    def lab(bi, s):
        try:
            nc._dbg_labels[bi.ins.name] = s
        except Exception:
            pass
        return bi
    xt_d = nc.declare_dram_parameter("xt", [NB, 128, 4096], bf16, isOutput=False)
    bc_d = nc.declare_dram_parameter("bc", [128, 896], bf16, isOutput=False)
    bn_d = nc.declare_dram_parameter("bn", [128, 896], bf16, isOutput=False)
    bn1_d = nc.declare_dram_parameter("bn1", [128, 1920], bf16, isOutput=False)
    btp_d = nc.declare_dram_parameter("btp", [128, 2, 1408], bf16, isOutput=False)
    btm_d = nc.declare_dram_parameter("btm", [128, 2, 1408], bf16, isOutput=False)
    out_d = nc.declare_dram_parameter("out", [B_SHARD, N], bf16, isOutput=True)

    with tile.TileContext(nc) as tc:
        with (
            tc.tile_pool(name="const", bufs=1) as constp,
            tc.tile_pool(name="xb", bufs=3) as xbp,
            tc.tile_pool(name="fold", bufs=2) as fp,
            tc.tile_pool(name="uf", bufs=2) as up,
            tc.tile_pool(name="psum", bufs=1, space="PSUM") as pp,
        ):
            bc = constp.tile([128, 896], bf16)
            bn = constp.tile([128, 896], bf16)
            bn1 = constp.tile([128, 1920], bf16)
            btp = constp.tile([128, 2, 1408], bf16)
            btm = constp.tile([128, 2, 1408], bf16)

            warm_in = constp.tile([128, 512], bf16, name="warm_in")
            nc.vector.memset(warm_in[:], 0.0)

            def emit_dma_in(blk):
                xbig = xbp.tile([128, 32, 128], bf16, tag="xbig", name="xbig")
                nc.sync.dma_start(xbig[:], xt_d[blk])
                return xbig

            def emit_tfolds(xbig, blk):
                xm = fp.tile([128, 16, 128], bf16, tag="xm", name="xm")
                lab(nc.vector.tensor_tensor(
                    xm[:], xbig[:, 16:32, :], xbig[:, 0:16, :], SUB
                ), f"b{blk}.xm")
                e = fp.tile([128, 4, 128], bf16, tag="e", name="e")
                lab(nc.vector.tensor_tensor(
                    e[:], xm[:, 8:12, :], xm[:, 0:4, :], ADD
                ), f"b{blk}.e")
                d = fp.tile([128, 4, 128], bf16, tag="d", name="d")
                lab(nc.vector.tensor_tensor(
                    d[:], xm[:, 12:16, :], xm[:, 4:8, :], SUB
                ), f"b{blk}.d")
                tpf = fp.tile([128, 8, 128], bf16, tag="tpf", name="tpf")
                tmf = fp.tile([128, 8, 128], bf16, tag="tmf", name="tmf")
                lab(nc.vector.scalar_tensor_tensor(
                    tpf[:, 0:4, :], e[:], ISQ2, xm[:, 4:8, :], MUL, ADD
                ), f"b{blk}.tpf1")
                lab(nc.vector.scalar_tensor_tensor(
                    tpf[:, 4:8, :], d[:], ISQ2, xm[:, 0:4, :], MUL, SUB
                ), f"b{blk}.tpf2")
                lab(nc.vector.scalar_tensor_tensor(
                    tmf[:, 0:4, :], e[:], ISQ2, xm[:, 4:8, :], MUL, SUB
                ), f"b{blk}.tmf1")
                lab(nc.vector.scalar_tensor_tensor(
                    tmf[:, 4:8, :], d[:], ISQ2, xm[:, 0:4, :], MUL, ADD
                ), f"b{blk}.tmf2")
                return tpf, tmf

            def emit_pfolds(xbig, blk):
                xp = fp.tile([128, 16, 128], bf16, tag="xp", name="xp")
                lab(nc.vector.tensor_tensor(
                    xp[:], xbig[:, 0:16, :], xbig[:, 16:32, :], ADD
                ), f"b{blk}.xpf")
                xpp = fp.tile([128, 8, 128], bf16, tag="xpp", name="xpp")
                lab(nc.vector.tensor_tensor(
                    xpp[:], xp[:, 8:16, :], xp[:, 0:8, :], ADD
                ), f"b{blk}.xpp")
                xpm = fp.tile([128, 8, 128], bf16, tag="xpm", name="xpm")
                lab(nc.vector.tensor_tensor(
                    xpm[:], xp[:, 8:16, :], xp[:, 0:8, :], SUB
                ), f"b{blk}.xpm")
                xc = fp.tile([128, 4, 128], bf16, tag="xc", name="xc")
                lab(nc.gpsimd.tensor_tensor(
                    xc[:], xpp[:, 4:8, :], xpp[:, 0:4, :], ADD
                ), f"b{blk}.xcf")
                xn = fp.tile([128, 4, 128], bf16, tag="xn", name="xn")
                lab(nc.gpsimd.tensor_tensor(
                    xn[:], xpp[:, 4:8, :], xpp[:, 0:4, :], SUB
                ), f"b{blk}.xnf")
                return xpm, xc, xn

            def mm_t(tAp, tAm, tpf, tmf, blk):
                for part, dst in ((0, "l"), (1, "h")):
                    for kk in range(8):
                        lab(nc.tensor.matmul(
                            tAp[:, 512 * part : 512 * part + 512], tpf[:, kk, :],
                            btp[:, part, 128 * kk : 128 * kk + 512],
                            start=(kk == 0), stop=(kk == 7),
                        ), f"b{blk}.tp{dst}{kk}")
                for part, dst in ((0, "l"), (1, "h")):
                    for kk in range(8):
                        lab(nc.tensor.matmul(
                            tAm[:, 512 * part : 512 * part + 512], tmf[:, kk, :],
                            btm[:, part, 128 * kk : 128 * kk + 512],
                            start=(kk == 0), stop=(kk == 7),
                        ), f"b{blk}.tm{dst}{kk}")

            def mm_b(tBn1, tBc, tBn, xpm, xc, xn, blk):
                for part, dst in ((0, "l"), (1, "h")):
                    for kk in range(8):
                        lab(nc.tensor.matmul(
                            tBn1[:, 512 * part : 512 * part + 512], xpm[:, kk, :],
                            bn1[:, 128 * kk + 512 * part : 128 * kk + 512 * part + 512],
                            start=(kk == 0), stop=(kk == 7),
                        ), f"b{blk}.n1{dst}{kk}")
                for kk in range(4):
                    lab(nc.tensor.matmul(
                        tBc[:], xc[:, kk, :],
                        bc[:, 128 * kk : 128 * kk + 512],
                        start=(kk == 0), stop=(kk == 3),
                    ), f"b{blk}.c{kk}")
                for kk in range(4):
                    lab(nc.tensor.matmul(
                        tBn[:], xn[:, kk, :],
                        bn[:, 128 * kk : 128 * kk + 512],
                        start=(kk == 0), stop=(kk == 3),
                    ), f"b{blk}.nn{kk}")

            def alloc_psum():
                tAp = pp.tile([128, 1024], f32, tag="tAp", name="tAp")
                tAm = pp.tile([128, 1024], f32, tag="tAm", name="tAm")
                tBn1 = pp.tile([128, 1024], f32, tag="tBn1", name="tBn1")
                tBc = pp.tile([128, 512], f32, tag="tBc", name="tBc")
                tBn = pp.tile([128, 512], f32, tag="tBn", name="tBn")
                return tAp, tAm, tBn1, tBc, tBn

            def emit_warm(tAp, n):
                for _ in range(n):
                    nc.tensor.matmul(
                        tAp[:, 0:512], warm_in[:, 0:128], warm_in[:],
                        start=True, stop=True,
                    )

            def emit_copies_ct(tAp, tAm, blk=0):
                ctp = up.tile([128, 1024], bf16, tag="ctp", name="ctp")
                ctm = up.tile([128, 1024], bf16, tag="ctm", name="ctm")
                lab(nc.scalar.mul(ctp[:], tAp[:], 1.0), f"b{blk}.Ctp")
                lab(nc.scalar.mul(ctm[:], tAm[:], 1.0), f"b{blk}.Ctm")
                return ctp, ctm

            def emit_copies_cs(tBn1, tBc, tBn, blk=0):
                cn1 = up.tile([128, 1024], bf16, tag="cn1", name="cn1")
                ccn = up.tile([128, 1024], bf16, tag="ccn", name="ccn")
                lab(nc.scalar.mul(cn1[:], tBn1[:], 1.0), f"b{blk}.Cn1")
                lab(nc.scalar.mul(ccn[:, 0:512], tBc[:], 1.0), f"b{blk}.Cc")
                lab(nc.scalar.mul(ccn[:, 512:1024], tBn[:], 1.0), f"b{blk}.Cn")
                return cn1, ccn

            def emit_unfold(blk, ctp, ctm, cn1, ccn):
                p1 = up.tile([128, 1024], bf16, tag="p1", name="p1")
                lab(nc.gpsimd.tensor_tensor(
                    p1[:, 0:512], ccn[:, 0:512], ccn[:, 512:1024], ADD
                ), f"u{blk}.p1l")
                lab(nc.gpsimd.tensor_tensor(
                    p1[:, 512:1024], ccn[:, 0:512], ccn[:, 512:1024], SUB
                ), f"u{blk}.p1h")
                p2 = up.tile([128, 2048], bf16, tag="p2", name="p2")
                lab(nc.gpsimd.tensor_tensor(
                    p2[:, 0:1024], p1[:], cn1[:], ADD
                ), f"u{blk}.p2l")
                lab(nc.gpsimd.tensor_tensor(
                    p2[:, 1024:2048], p1[:], cn1[:], SUB
                ), f"u{blk}.p2h")
                m2 = up.tile([128, 2048], bf16, tag="m2", name="m2")
                lab(nc.vector.tensor_tensor(
                    m2[:, 1536:2048], ctm[:, 0:512], ctp[:, 0:512], SUB
                ), f"u{blk}.m2bh")
                lab(nc.vector.tensor_tensor(
                    m2[:, 1024:1536], ctp[:, 512:1024], ctm[:, 512:1024], SUB
                ), f"u{blk}.m2bl")
                dtet = up.tile([128, 1024], bf16, tag="dtet", name="dtet")
                lab(nc.vector.tensor_tensor(
                    dtet[:], ctp[:], ctm[:], ADD
                ), f"u{blk}.dtet")
                lab(nc.vector.scalar_tensor_tensor(
                    m2[:, 0:512], dtet[:, 0:512], SQ2, m2[:, 1024:1536], MUL, ADD
                ), f"u{blk}.m2al")
                lab(nc.vector.scalar_tensor_tensor(
                    m2[:, 512:1024], dtet[:, 512:1024], SQ2, m2[:, 1536:2048],
                    MUL, SUB
                ), f"u{blk}.m2ah")
                olo = up.tile([128, 2048], bf16, tag="olo", name="olo")
                lab(nc.vector.tensor_tensor(olo[:], p2[:], m2[:], ADD),
                    f"u{blk}.olo")
                nc.sync.dma_start(out_d[128 * blk : 128 * blk + 128, 0:2048], olo[:])
                ohi = up.tile([128, 2048], bf16, tag="ohi", name="ohi")
                lab(nc.vector.tensor_tensor(ohi[:], p2[:], m2[:], SUB),
                    f"u{blk}.ohi")
                nc.sync.dma_start(
                    out_d[128 * blk : 128 * blk + 128, 2048:4096], ohi[:]
                )

            xbigs = {0: emit_dma_in(0)}
            nc.sync.dma_start(btp[:, 0, :], btp_d[:, 0, :])
            nc.sync.dma_start(btp[:, 1, :], btp_d[:, 1, :])
            nc.sync.dma_start(btm[:, 0, :], btm_d[:, 0, :])
            nc.sync.dma_start(btm[:, 1, :], btm_d[:, 1, :])
            nc.sync.dma_start(bn1[:], bn1_d[:])
            nc.sync.dma_start(bc[:], bc_d[:])
            nc.sync.dma_start(bn[:], bn_d[:])
            xbigs[1] = emit_dma_in(1)

            tf = {0: emit_tfolds(xbigs[0], 0)}
            pf = {0: emit_pfolds(xbigs[0], 0)}
            tAp, tAm, tBn1, tBc, tBn = alloc_psum()
            emit_warm(tAp, 13)
            tpf, tmf = tf.pop(0)
            xpm, xc, xn = pf.pop(0)
            mm_t(tAp, tAm, tpf, tmf, 0)
            mm_b(tBn1, tBc, tBn, xpm, xc, xn, 0)
            ct_ = {0: emit_copies_ct(tAp, tAm, 0)}
            xbigs[2] = emit_dma_in(2)
            tf[1] = emit_tfolds(xbigs[1], 1)
            cs_ = {0: emit_copies_cs(tBn1, tBc, tBn, 0)}
            pf[1] = emit_pfolds(xbigs.pop(1), 1)

            for i in range(1, NB - 1):
                tpf, tmf = tf.pop(i)
                xpm, xc, xn = pf.pop(i)
                tAp, tAm, tBn1, tBc, tBn = alloc_psum()
                mm_t(tAp, tAm, tpf, tmf, i)
                mm_b(tBn1, tBc, tBn, xpm, xc, xn, i)
                ct_[i] = emit_copies_ct(tAp, tAm, i)
                tf[i + 1] = emit_tfolds(xbigs[i + 1], i + 1)
                cs_[i] = emit_copies_cs(tBn1, tBc, tBn, i)
                emit_unfold(i - 1, *ct_.pop(i - 1), *cs_.pop(i - 1))
                pf[i + 1] = emit_pfolds(xbigs.pop(i + 1), i + 1)
                if i + 2 < NB:
                    xbigs[i + 2] = emit_dma_in(i + 2)

            i = NB - 1
            b0 = 128 * i
            tpf, tmf = tf.pop(i)
            xpm, xc, xn = pf.pop(i)
            emit_unfold(NB - 2, *ct_.pop(NB - 2), *cs_.pop(NB - 2))
            tAp, tAm, tBn1, tBc, tBn = alloc_psum()
            mm_t(tAp, tAm, tpf, tmf, 99)
            ctp, ctm = emit_copies_ct(tAp, tAm, 99)
            mm_b(tBn1, tBc, tBn, xpm, xc, xn, 99)
            m2 = up.tile([128, 2048], bf16, tag="m2", name="m2")
            lab(nc.vector.tensor_tensor(
                m2[:, 1536:2048], ctm[:, 0:512], ctp[:, 0:512], SUB
            ), "t.m2bh")
            lab(nc.vector.tensor_tensor(
                m2[:, 1024:1536], ctp[:, 512:1024], ctm[:, 512:1024], SUB
            ), "t.m2bl")
            dtet = up.tile([128, 1024], bf16, tag="dtet", name="dtet")
            lab(nc.vector.tensor_tensor(dtet[:], ctp[:], ctm[:], ADD), "t.dtet")
            lab(nc.vector.scalar_tensor_tensor(
                m2[:, 0:512], dtet[:, 0:512], SQ2, m2[:, 1024:1536], MUL, ADD
            ), "t.m2al")
            lab(nc.vector.scalar_tensor_tensor(
                m2[:, 512:1024], dtet[:, 512:1024], SQ2, m2[:, 1536:2048], MUL, SUB
            ), "t.m2ah")
            cn1 = up.tile([128, 1024], bf16, tag="cn1", name="cn1")
            lab(nc.scalar.mul(cn1[:], tBn1[:], 1.0), "t.Cn1")
            ccn = up.tile([128, 1024], bf16, tag="ccn", name="ccn")
            lab(nc.scalar.mul(ccn[:, 0:512], tBc[:], 1.0), "t.Cc")
            lab(nc.scalar.mul(ccn[:, 512:1024], tBn[:], 1.0), "t.Cn")
            p1 = up.tile([128, 1024], bf16, tag="p1", name="p1")
            lab(nc.vector.tensor_tensor(
                p1[:, 0:512], ccn[:, 0:512], ccn[:, 512:1024], ADD
            ), "t.p1l")
            lab(nc.vector.tensor_tensor(
                p1[:, 512:1024], ccn[:, 0:512], ccn[:, 512:1024], SUB
            ), "t.p1h")
            p2 = up.tile([128, 2048], bf16, tag="p2", name="p2")
            lab(nc.vector.tensor_tensor(p2[:, 0:1024], p1[:], cn1[:], ADD),
                "t.p2l")
            olo = up.tile([128, 2048], bf16, tag="olo", name="olo")
            ohi = up.tile([128, 2048], bf16, tag="ohi", name="ohi")
            lab(nc.vector.tensor_tensor(
                olo[:, 0:1024], p2[:, 0:1024], m2[:, 0:1024], ADD
            ), "t.olol")
            nc.sync.dma_start(out_d[b0 : b0 + 128, 0:1024], olo[:, 0:1024])
            lab(nc.vector.tensor_tensor(p2[:, 1024:2048], p1[:], cn1[:], SUB),
                "t.p2h")
            lab(nc.vector.tensor_tensor(
                olo[:, 1024:2048], p2[:, 1024:2048], m2[:, 1024:2048], ADD
            ), "t.olor")
            nc.sync.dma_start(out_d[b0 : b0 + 128, 1024:2048], olo[:, 1024:2048])
            lab(nc.vector.tensor_tensor(
                ohi[:, 0:1024], p2[:, 0:1024], m2[:, 0:1024], SUB
            ), "t.ohil")
            nc.sync.dma_start(out_d[b0 : b0 + 128, 2048:3072], ohi[:, 0:1024])
            lab(nc.vector.tensor_tensor(
                ohi[:, 1024:2048], p2[:, 1024:2048], m2[:, 1024:2048], SUB
            ), "t.ohir")
            nc.sync.dma_start(out_d[b0 : b0 + 128, 3072:4096], ohi[:, 1024:2048])

    nc.compile()
    return nc


def _get_nc():
    if "nc" not in _STATE:
        _STATE["nc"] = _build()
    return _STATE["nc"]


def _pad_slice(a, lo, hi):
    out = np.zeros(hi - lo)
    s, e = max(0, lo), min(len(a), hi)
    if e > s:
        out[s - lo : e - lo] = a[s:e]
    return out


def _build_bands(w):
    W0 = np.asarray(w, dtype=np.float64)
    W = np.roll(W0[::-1], 1)
    SQ = np.sqrt(2.0)
    g_c = 1.0 / 8.0
    g_n1 = 1.0 / 4.0
    g_t = 1.0 / (4.0 * SQ)

    Wp2048 = W[:2048] + W[2048:]
    Wm2048 = W[:2048] - W[2048:]
    Wp1024 = Wp2048[:1024] + Wp2048[1024:]
    Wn1024 = Wp2048[:1024] - Wp2048[1024:]
    Wc512 = Wp1024[:512] + Wp1024[512:]
    Wn512 = Wp1024[:512] - Wp1024[512:]

    def tri_reduce(P, alpha):
        mc = len(P) // 2
        A, Bb = P[:mc], P[mc:]
        h = mc // 2
        lo = A[:h] - Bb[:h] - alpha * Bb[h:]
        hi = A[h:] + alpha * Bb[:h] + (alpha * alpha - 1.0) * Bb[h:]
        return np.concatenate([lo, hi])

    def tri_G(WT, m, alpha):
        h = m // 2
        width = m + h - 1
        Glo = (
            _pad_slice(WT, 1 - m, 1 - m + width)
            - _pad_slice(WT, 1, 1 + width)
            - alpha * _pad_slice(WT, 1 + h, 1 + h + width)
        )
        Ghi = (
            _pad_slice(WT, 1 - h, 1 - h + width)
            + alpha * _pad_slice(WT, 1, 1 + width)
            + (alpha * alpha - 1.0) * _pad_slice(WT, 1 + h, 1 + h + width)
        )
        return Glo, Ghi

    def shear(G, ncols):
        Gp = np.zeros(127 + ncols)
        n = min(len(G), 127 + ncols)
        Gp[:n] = G[:n]
        return np.lib.stride_tricks.sliding_window_view(Gp, ncols)[:128].copy()

    def shear2(Glo, Ghi, ncols):
        return np.stack([shear(Glo, ncols), shear(Ghi, ncols)], axis=1)

    t = np.arange(1023)
    Gc = g_c * Wc512[(1 + t) % 512]
    v = t - 511
    Gn = g_c * np.where(v >= 0, 1.0, -1.0) * Wn512[v % 512]
    t1 = np.arange(2047)
    v1 = t1 - 1023
    Gn1 = g_n1 * np.where(v1 >= 0, 1.0, -1.0) * Wn1024[v1 % 1024]
    WTp = tri_reduce(Wm2048, SQ)
    WTm = tri_reduce(Wm2048, -SQ)
    bands = {
        "bc": shear(Gc, 896),
        "bn": shear(Gn, 896),
        "bn1": shear(Gn1, 1920),
        "btp": (g_t * SQ) * shear2(*tri_G(WTp, 1024, SQ), 1408),
        "btm": (g_t * SQ) * shear2(*tri_G(WTm, 1024, -SQ), 1408),
    }
    return {
        k: np.ascontiguousarray(v, dtype=ml_dtypes.bfloat16)
        for k, v in bands.items()
    }


def _prep_inputs(x, w):
    x = np.asarray(x, dtype=np.float32)
    bands = _build_bands(np.asarray(w, dtype=np.float64))
    in_maps = []
    for i in range(N_CORES):
        xs = x[i * B_SHARD : (i + 1) * B_SHARD]
        X = xs[:, ::-1].T
        X4 = X.reshape(32, 128, NB, 128)
        xt = np.ascontiguousarray(
            X4.transpose(2, 1, 0, 3).reshape(NB, 128, 4096),
            dtype=ml_dtypes.bfloat16,
        )
        in_maps.append({"xt": xt, **bands})
    return in_maps


def kernel(x, w, _trace=False):
    from concourse.bass_utils import run_bass_kernel_spmd

    nc = _get_nc()
    in_maps = _prep_inputs(x, w)
    res = run_bass_kernel_spmd(nc, in_maps, list(range(N_CORES)), trace=_trace)
    out = np.concatenate(
        [res.results[i]["out"].astype(np.float32) for i in range(N_CORES)], axis=0
    )
    if _trace:
        _STATE["last_result"] = res
    return out


# revision 22
# speedup vs baseline: 1.5511x; 1.0078x over previous
import sys

sys.path.insert(0, "/opt/trn_rl_repo")

import numpy as np
import ml_dtypes

N = 4096
B = 8192
N_CORES = 8
B_SHARD = B // N_CORES
NB = B_SHARD // 128
SQ2 = float(np.sqrt(2.0))
ISQ2 = float(1.0 / np.sqrt(2.0))

_STATE = {}


def _build():
    import concourse.bacc as bacc
    import concourse.mybir as mybir
    import concourse.tile as tile

    f32 = mybir.dt.float32
    bf16 = mybir.dt.bfloat16
    ADD = mybir.AluOpType.add
    SUB = mybir.AluOpType.subtract
    MUL = mybir.AluOpType.mult

    nc = bacc.Bacc("TRN2", target_bir_lowering=False, debug=False)
    nc._dbg_labels = {}

    def lab(bi, s):
        try:
            nc._dbg_labels[bi.ins.name] = s
        except Exception:
            pass
        return bi

    xt_d = nc.declare_dram_parameter("xt", [NB, 128, 4096], bf16, isOutput=False)
    bc_d = nc.declare_dram_parameter("bc", [128, 896], bf16, isOutput=False)
    bn_d = nc.declare_dram_parameter("bn", [128, 896], bf16, isOutput=False)
    bn1_d = nc.declare_dram_parameter("bn1", [128, 1920], bf16, isOutput=False)
    btp_d = nc.declare_dram_parameter("btp", [128, 2, 1408], bf16, isOutput=False)
    btm_d = nc.declare_dram_parameter("btm", [128, 2, 1408], bf16, isOutput=False)
    out_d = nc.declare_dram_parameter("out", [B_SHARD, N], bf16, isOutput=True)

    with tile.TileContext(nc) as tc:
        with (
            tc.tile_pool(name="const", bufs=1) as constp,
            tc.tile_pool(name="xb", bufs=3) as xbp,
            tc.tile_pool(name="fold", bufs=2) as fp,
            tc.tile_pool(name="uf", bufs=2) as up,
            tc.tile_pool(name="psum", bufs=1, space="PSUM") as pp,
        ):
            bc = constp.tile([128, 896], bf16)
            bn = constp.tile([128, 896], bf16)
            bn1 = constp.tile([128, 1920], bf16)
            btp = constp.tile([128, 2, 1408], bf16)
            btm = constp.tile([128, 2, 1408], bf16)

            warm_in = constp.tile([128, 512], bf16, name="warm_in")
            nc.vector.memset(warm_in[:], 0.0)

            def emit_dma_in(blk):
                xbig = xbp.tile([128, 32, 128], bf16, tag="xbig", name="xbig")
                nc.sync.dma_start(xbig[:], xt_d[blk])
                return xbig

            def emit_tfolds(xbig, blk):
                xm = fp.tile([128, 16, 128], bf16, tag="xm", name="xm")
                lab(nc.vector.tensor_tensor(
                    xm[:], xbig[:, 16:32, :], xbig[:, 0:16, :], SUB
                ), f"b{blk}.xm")
                e = fp.tile([128, 4, 128], bf16, tag="e", name="e")
                lab(nc.vector.tensor_tensor(
                    e[:], xm[:, 8:12, :], xm[:, 0:4, :], ADD
                ), f"b{blk}.e")
                d = fp.tile([128, 4, 128], bf16, tag="d", name="d")
                lab(nc.vector.tensor_tensor(
                    d[:], xm[:, 12:16, :], xm[:, 4:8, :], SUB
                ), f"b{blk}.d")
                tpfa = fp.tile([128, 4, 128], bf16, tag="tpfa", name="tpfa")
                tpfb = fp.tile([128, 4, 128], bf16, tag="tpfb", name="tpfb")
                tmfa = fp.tile([128, 4, 128], bf16, tag="tmfa", name="tmfa")
                tmfb = fp.tile([128, 4, 128], bf16, tag="tmfb", name="tmfb")
                lab(nc.vector.scalar_tensor_tensor(
                    tpfa[:], e[:], ISQ2, xm[:, 4:8, :], MUL, ADD
                ), f"b{blk}.tpf1")
                lab(nc.vector.scalar_tensor_tensor(
                    tpfb[:], d[:], ISQ2, xm[:, 0:4, :], MUL, SUB
                ), f"b{blk}.tpf2")
                lab(nc.vector.scalar_tensor_tensor(
                    tmfa[:], e[:], ISQ2, xm[:, 4:8, :], MUL, SUB
                ), f"b{blk}.tmf1")
                lab(nc.vector.scalar_tensor_tensor(
                    tmfb[:], d[:], ISQ2, xm[:, 0:4, :], MUL, ADD
                ), f"b{blk}.tmf2")
                return (tpfa, tpfb), (tmfa, tmfb)

            def emit_pfolds(xbig, blk):
                xp = fp.tile([128, 16, 128], bf16, tag="xp", name="xp")
                lab(nc.vector.tensor_tensor(
                    xp[:], xbig[:, 0:16, :], xbig[:, 16:32, :], ADD
                ), f"b{blk}.xpf")
                xpp = fp.tile([128, 8, 128], bf16, tag="xpp", name="xpp")
                lab(nc.vector.tensor_tensor(
                    xpp[:], xp[:, 8:16, :], xp[:, 0:8, :], ADD
                ), f"b{blk}.xpp")
                xpm = fp.tile([128, 8, 128], bf16, tag="xpm", name="xpm")
                lab(nc.vector.tensor_tensor(
                    xpm[:], xp[:, 8:16, :], xp[:, 0:8, :], SUB
                ), f"b{blk}.xpm")
                xc = fp.tile([128, 4, 128], bf16, tag="xc", name="xc")
                lab(nc.gpsimd.tensor_tensor(
                    xc[:], xpp[:, 4:8, :], xpp[:, 0:4, :], ADD
                ), f"b{blk}.xcf")
                xn = fp.tile([128, 4, 128], bf16, tag="xn", name="xn")
                lab(nc.gpsimd.tensor_tensor(
                    xn[:], xpp[:, 4:8, :], xpp[:, 0:4, :], SUB
                ), f"b{blk}.xnf")
                return xpm, xc, xn

            def alloc_psum():
                tAp = pp.tile([128, 1024], f32, tag="tAp", name="tAp")
                tAm = pp.tile([128, 1024], f32, tag="tAm", name="tAm")
                tBn1 = pp.tile([128, 1024], f32, tag="tBn1", name="tBn1")
                tBc = pp.tile([128, 512], f32, tag="tBc", name="tBc")
                tBn = pp.tile([128, 512], f32, tag="tBn", name="tBn")
                return tAp, tAm, tBn1, tBc, tBn

            def mm_t(tAp, tAm, tpf, tmf, blk):
                for part, dst in ((0, "l"), (1, "h")):
                    for kk in range(8):
                        st = tpf[kk // 4][:, kk % 4, :]
                        lab(nc.tensor.matmul(
                            tAp[:, 512 * part : 512 * part + 512], st,
                            btp[:, part, 128 * kk : 128 * kk + 512],
                            start=(kk == 0), stop=(kk == 7),
                        ), f"b{blk}.tp{dst}{kk}")
                for part, dst in ((0, "l"), (1, "h")):
                    for kk in range(8):
                        st = tmf[kk // 4][:, kk % 4, :]
                        lab(nc.tensor.matmul(
                            tAm[:, 512 * part : 512 * part + 512], st,
                            btm[:, part, 128 * kk : 128 * kk + 512],
                            start=(kk == 0), stop=(kk == 7),
                        ), f"b{blk}.tm{dst}{kk}")

            def mm_cn(tBc, tBn, xc, xn, blk):
                for kk in range(4):
                    lab(nc.tensor.matmul(
                        tBc[:], xc[:, kk, :],
                        bc[:, 128 * kk : 128 * kk + 512],
                        start=(kk == 0), stop=(kk == 3),
                    ), f"b{blk}.c{kk}")
                for kk in range(4):
                    lab(nc.tensor.matmul(
                        tBn[:], xn[:, kk, :],
                        bn[:, 128 * kk : 128 * kk + 512],
                        start=(kk == 0), stop=(kk == 3),
                    ), f"b{blk}.nn{kk}")

            def mm_n1(tBn1, xpm, blk):
                for part, dst in ((0, "l"), (1, "h")):
                    for kk in range(8):
                        lab(nc.tensor.matmul(
                            tBn1[:, 512 * part : 512 * part + 512], xpm[:, kk, :],
                            bn1[:, 128 * kk + 512 * part :
                                 128 * kk + 512 * part + 512],
                            start=(kk == 0), stop=(kk == 7),
                        ), f"b{blk}.n1{dst}{kk}")

            def emit_warm(tAp, n):
                for _ in range(n):
                    nc.tensor.matmul(
                        tAp[:, 0:512], warm_in[:, 0:128], warm_in[:],
                        start=True, stop=True,
                    )

            def emit_copies_ct(tAp, tAm, blk=0):
                ctp = up.tile([128, 1024], bf16, tag="ctp", name="ctp")
                ctm = up.tile([128, 1024], bf16, tag="ctm", name="ctm")
                lab(nc.scalar.mul(ctp[:], tAp[:], 1.0), f"b{blk}.Ctp")
                lab(nc.scalar.mul(ctm[:], tAm[:], 1.0), f"b{blk}.Ctm")
                return ctp, ctm

            def emit_copies_cs(tBn1, tBc, tBn, blk=0):
                cn1 = up.tile([128, 1024], bf16, tag="cn1", name="cn1")
                ccn = up.tile([128, 1024], bf16, tag="ccn", name="ccn")
                lab(nc.scalar.mul(cn1[:], tBn1[:], 1.0), f"b{blk}.Cn1")
                lab(nc.scalar.mul(ccn[:, 0:512], tBc[:], 1.0), f"b{blk}.Cc")
                lab(nc.scalar.mul(ccn[:, 512:1024], tBn[:], 1.0), f"b{blk}.Cn")
                return cn1, ccn

            def emit_unfold(blk, ctp, ctm, cn1, ccn):
                p1 = up.tile([128, 1024], bf16, tag="p1", name="p1")
                lab(nc.gpsimd.tensor_tensor(
                    p1[:, 0:512], ccn[:, 0:512], ccn[:, 512:1024], ADD
                ), f"u{blk}.p1l")
                lab(nc.gpsimd.tensor_tensor(
                    p1[:, 512:1024], ccn[:, 0:512], ccn[:, 512:1024], SUB
                ), f"u{blk}.p1h")
                p2 = up.tile([128, 2048], bf16, tag="p2", name="p2")
                lab(nc.gpsimd.tensor_tensor(
                    p2[:, 0:1024], p1[:], cn1[:], ADD
                ), f"u{blk}.p2l")
                lab(nc.gpsimd.tensor_tensor(
                    p2[:, 1024:2048], p1[:], cn1[:], SUB
                ), f"u{blk}.p2h")
                m2 = up.tile([128, 2048], bf16, tag="m2", name="m2")
                lab(nc.vector.tensor_tensor(
                    m2[:, 1536:2048], ctm[:, 0:512], ctp[:, 0:512], SUB
                ), f"u{blk}.m2bh")
                lab(nc.vector.tensor_tensor(
                    m2[:, 1024:1536], ctp[:, 512:1024], ctm[:, 512:1024], SUB
                ), f"u{blk}.m2bl")
                dtet = up.tile([128, 1024], bf16, tag="dtet", name="dtet")
                lab(nc.vector.tensor_tensor(
                    dtet[:], ctp[:], ctm[:], ADD
                ), f"u{blk}.dtet")
                dtets = up.tile([128, 1024], bf16, tag="dtets", name="dtets")
                lab(nc.scalar.mul(dtets[:], dtet[:], SQ2), f"u{blk}.dtets")
                lab(nc.vector.tensor_tensor(
                    m2[:, 0:512], dtets[:, 0:512], m2[:, 1024:1536], ADD
                ), f"u{blk}.m2al")
                lab(nc.vector.tensor_tensor(
                    m2[:, 512:1024], dtets[:, 512:1024], m2[:, 1536:2048], SUB
                ), f"u{blk}.m2ah")
                olo = up.tile([128, 2048], bf16, tag="olo", name="olo")
                lab(nc.vector.tensor_tensor(olo[:], p2[:], m2[:], ADD),
                    f"u{blk}.olo")
                nc.sync.dma_start(out_d[128 * blk : 128 * blk + 128, 0:2048], olo[:])
                ohi = up.tile([128, 2048], bf16, tag="ohi", name="ohi")
                lab(nc.vector.tensor_tensor(ohi[:], p2[:], m2[:], SUB),
                    f"u{blk}.ohi")
                nc.sync.dma_start(
                    out_d[128 * blk : 128 * blk + 128, 2048:4096], ohi[:]
                )

            xbigs = {0: emit_dma_in(0)}
            nc.sync.dma_start(btp[:, 0, :], btp_d[:, 0, :])
            nc.sync.dma_start(btp[:, 1, :], btp_d[:, 1, :])
            nc.sync.dma_start(btm[:, 0, :], btm_d[:, 0, :])
            nc.sync.dma_start(btm[:, 1, :], btm_d[:, 1, :])
            nc.sync.dma_start(bn1[:], bn1_d[:])
            nc.sync.dma_start(bc[:], bc_d[:])
            nc.sync.dma_start(bn[:], bn_d[:])
            xbigs[1] = emit_dma_in(1)

            tf = {0: emit_tfolds(xbigs[0], 0)}
            pf = {0: emit_pfolds(xbigs[0], 0)}
            tAp, tAm, tBn1, tBc, tBn = alloc_psum()
            emit_warm(tAp, 16)
            tpf, tmf = tf.pop(0)
            xpm, xc, xn = pf.pop(0)
            mm_t(tAp, tAm, tpf, tmf, 0)
            mm_n1(tBn1, xpm, 0)
            mm_cn(tBc, tBn, xc, xn, 0)
            ct_ = {0: emit_copies_ct(tAp, tAm, 0)}
            xbigs[2] = emit_dma_in(2)
            tf[1] = emit_tfolds(xbigs[1], 1)
            cs_ = {0: emit_copies_cs(tBn1, tBc, tBn, 0)}
            pf[1] = emit_pfolds(xbigs.pop(1), 1)

            for i in range(1, NB - 1):
                tpf, tmf = tf.pop(i)
                xpm, xc, xn = pf.pop(i)
                tAp, tAm, tBn1, tBc, tBn = alloc_psum()
                mm_t(tAp, tAm, tpf, tmf, i)
                mm_n1(tBn1, xpm, i)
                mm_cn(tBc, tBn, xc, xn, i)
                ct_[i] = emit_copies_ct(tAp, tAm, i)
                tf[i + 1] = emit_tfolds(xbigs[i + 1], i + 1)
                cs_[i] = emit_copies_cs(tBn1, tBc, tBn, i)
                emit_unfold(i - 1, *ct_.pop(i - 1), *cs_.pop(i - 1))
                pf[i + 1] = emit_pfolds(xbigs.pop(i + 1), i + 1)
                if i + 2 < NB:
                    xbigs[i + 2] = emit_dma_in(i + 2)

            i = NB - 1
            b0 = 128 * i
            tpf, tmf = tf.pop(i)
            xpm, xc, xn = pf.pop(i)
            emit_unfold(NB - 2, *ct_.pop(NB - 2), *cs_.pop(NB - 2))
            tAp, tAm, tBn1, tBc, tBn = alloc_psum()
            mm_t(tAp, tAm, tpf, tmf, 99)
            ctp, ctm = emit_copies_ct(tAp, tAm, 99)
            mm_cn(tBc, tBn, xc, xn, 99)
            ccn = up.tile([128, 1024], bf16, tag="ccn", name="ccn")
            lab(nc.scalar.mul(ccn[:, 0:512], tBc[:], 1.0), "t.Cc")
            lab(nc.scalar.mul(ccn[:, 512:1024], tBn[:], 1.0), "t.Cn")
            mm_n1(tBn1, xpm, 99)
            m2 = up.tile([128, 2048], bf16, tag="m2", name="m2")
            lab(nc.vector.tensor_tensor(
                m2[:, 1536:2048], ctm[:, 0:512], ctp[:, 0:512], SUB
            ), "t.m2bh")
            lab(nc.vector.tensor_tensor(
                m2[:, 1024:1536], ctp[:, 512:1024], ctm[:, 512:1024], SUB
            ), "t.m2bl")
            dtet = up.tile([128, 1024], bf16, tag="dtet", name="dtet")
            lab(nc.vector.tensor_tensor(dtet[:], ctp[:], ctm[:], ADD), "t.dtet")
            lab(nc.vector.scalar_tensor_tensor(
                m2[:, 0:512], dtet[:, 0:512], SQ2, m2[:, 1024:1536], MUL, ADD
            ), "t.m2al")
            lab(nc.vector.scalar_tensor_tensor(
                m2[:, 512:1024], dtet[:, 512:1024], SQ2, m2[:, 1536:2048], MUL, SUB
            ), "t.m2ah")
            p1 = up.tile([128, 1024], bf16, tag="p1", name="p1")
            lab(nc.vector.tensor_tensor(
                p1[:, 0:512], ccn[:, 0:512], ccn[:, 512:1024], ADD
            ), "t.p1l")
            lab(nc.vector.tensor_tensor(
                p1[:, 512:1024], ccn[:, 0:512], ccn[:, 512:1024], SUB
            ), "t.p1h")
            cn1 = up.tile([128, 1024], bf16, tag="cn1", name="cn1")
            lab(nc.scalar.mul(cn1[:], tBn1[:], 1.0), "t.Cn1")
            p2 = up.tile([128, 2048], bf16, tag="p2", name="p2")
            lab(nc.vector.tensor_tensor(p2[:, 0:1024], p1[:], cn1[:], ADD),
                "t.p2l")
            olo = up.tile([128, 2048], bf16, tag="olo", name="olo")
            ohi = up.tile([128, 2048], bf16, tag="ohi", name="ohi")
            lab(nc.vector.tensor_tensor(
                olo[:, 0:1024], p2[:, 0:1024], m2[:, 0:1024], ADD
            ), "t.olol")
            nc.sync.dma_start(out_d[b0 : b0 + 128, 0:1024], olo[:, 0:1024])
            lab(nc.vector.tensor_tensor(p2[:, 1024:2048], p1[:], cn1[:], SUB),
                "t.p2h")
            lab(nc.vector.tensor_tensor(
                olo[:, 1024:2048], p2[:, 1024:2048], m2[:, 1024:2048], ADD
            ), "t.olor")
            nc.sync.dma_start(out_d[b0 : b0 + 128, 1024:2048], olo[:, 1024:2048])
            lab(nc.vector.tensor_tensor(
                ohi[:, 0:1024], p2[:, 0:1024], m2[:, 0:1024], SUB
            ), "t.ohil")
            nc.sync.dma_start(out_d[b0 : b0 + 128, 2048:3072], ohi[:, 0:1024])
            lab(nc.vector.tensor_tensor(
                ohi[:, 1024:2048], p2[:, 1024:2048], m2[:, 1024:2048], SUB
            ), "t.ohir")
            nc.sync.dma_start(out_d[b0 : b0 + 128, 3072:4096], ohi[:, 1024:2048])

    nc.compile()
    return nc


def _get_nc():
    if "nc" not in _STATE:
        _STATE["nc"] = _build()
    return _STATE["nc"]


def _pad_slice(a, lo, hi):
    out = np.zeros(hi - lo)
    s, e = max(0, lo), min(len(a), hi)
    if e > s:
        out[s - lo : e - lo] = a[s:e]
    return out


def _build_bands(w):
    W0 = np.asarray(w, dtype=np.float64)
    W = np.roll(W0[::-1], 1)
    SQ = np.sqrt(2.0)
    g_c = 1.0 / 8.0
    g_n1 = 1.0 / 4.0
    g_t = 1.0 / (4.0 * SQ)

    Wp2048 = W[:2048] + W[2048:]
    Wm2048 = W[:2048] - W[2048:]
    Wp1024 = Wp2048[:1024] + Wp2048[1024:]
    Wn1024 = Wp2048[:1024] - Wp2048[1024:]
    Wc512 = Wp1024[:512] + Wp1024[512:]
    Wn512 = Wp1024[:512] - Wp1024[512:]

    def tri_reduce(P, alpha):
        mc = len(P) // 2
        A, Bb = P[:mc], P[mc:]
        h = mc // 2
        lo = A[:h] - Bb[:h] - alpha * Bb[h:]
        hi = A[h:] + alpha * Bb[:h] + (alpha * alpha - 1.0) * Bb[h:]
        return np.concatenate([lo, hi])

    def tri_G(WT, m, alpha):
        h = m // 2
        width = m + h - 1
        Glo = (
            _pad_slice(WT, 1 - m, 1 - m + width)
            - _pad_slice(WT, 1, 1 + width)
            - alpha * _pad_slice(WT, 1 + h, 1 + h + width)
        )
        Ghi = (
            _pad_slice(WT, 1 - h, 1 - h + width)
            + alpha * _pad_slice(WT, 1, 1 + width)
            + (alpha * alpha - 1.0) * _pad_slice(WT, 1 + h, 1 + h + width)
        )
        return Glo, Ghi

    def shear(G, ncols):
        Gp = np.zeros(127 + ncols)
        n = min(len(G), 127 + ncols)
        Gp[:n] = G[:n]
        return np.lib.stride_tricks.sliding_window_view(Gp, ncols)[:128].copy()

    def shear2(Glo, Ghi, ncols):
        return np.stack([shear(Glo, ncols), shear(Ghi, ncols)], axis=1)

    t = np.arange(1023)
    Gc = g_c * Wc512[(1 + t) % 512]
    v = t - 511
    Gn = g_c * np.where(v >= 0, 1.0, -1.0) * Wn512[v % 512]
    t1 = np.arange(2047)
    v1 = t1 - 1023
    Gn1 = g_n1 * np.where(v1 >= 0, 1.0, -1.0) * Wn1024[v1 % 1024]
    WTp = tri_reduce(Wm2048, SQ)
    WTm = tri_reduce(Wm2048, -SQ)
    bands = {
        "bc": shear(Gc, 896),
        "bn": shear(Gn, 896),
        "bn1": shear(Gn1, 1920),
        "btp": (g_t * SQ) * shear2(*tri_G(WTp, 1024, SQ), 1408),
        "btm": (g_t * SQ) * shear2(*tri_G(WTm, 1024, -SQ), 1408),
    }
    return {
        k: np.ascontiguousarray(v, dtype=ml_dtypes.bfloat16)
        for k, v in bands.items()
    }


def _prep_inputs(x, w):
    x = np.asarray(x, dtype=np.float32)
    bands = _build_bands(np.asarray(w, dtype=np.float64))
    in_maps = []
    for i in range(N_CORES):
        xs = x[i * B_SHARD : (i + 1) * B_SHARD]
        X = xs[:, ::-1].T
        X4 = X.reshape(32, 128, NB, 128)
        xt = np.ascontiguousarray(
            X4.transpose(2, 1, 0, 3).reshape(NB, 128, 4096),
            dtype=ml_dtypes.bfloat16,
        )
        in_maps.append({"xt": xt, **bands})
    return in_maps


def kernel(x, w, _trace=False):
    from concourse.bass_utils import run_bass_kernel_spmd

    nc = _get_nc()
    in_maps = _prep_inputs(x, w)
    res = run_bass_kernel_spmd(nc, in_maps, list(range(N_CORES)), trace=_trace)
    out = np.concatenate(
        [res.results[i]["out"].astype(np.float32) for i in range(N_CORES)], axis=0
    )
    if _trace:
        _STATE["last_result"] = res
    return out


# revision 26
# speedup vs baseline: 1.5569x; 1.0038x over previous
import sys

sys.path.insert(0, "/opt/trn_rl_repo")

import numpy as np
import ml_dtypes

N = 4096
B = 8192
N_CORES = 8
B_SHARD = B // N_CORES
NB = B_SHARD // 128
SQ2 = float(np.sqrt(2.0))
ISQ2 = float(1.0 / np.sqrt(2.0))

_STATE = {}


def _build():
    import concourse.bacc as bacc
    import concourse.mybir as mybir
    import concourse.tile as tile

    f32 = mybir.dt.float32
    bf16 = mybir.dt.bfloat16
    ADD = mybir.AluOpType.add
    SUB = mybir.AluOpType.subtract
    MUL = mybir.AluOpType.mult

    nc = bacc.Bacc("TRN2", target_bir_lowering=False, debug=False)
    nc._dbg_labels = {}

    def lab(bi, s):
        try:
            nc._dbg_labels[bi.ins.name] = s
        except Exception:
            pass
        return bi

    xt_d = nc.declare_dram_parameter("xt", [NB, 128, 4096], bf16, isOutput=False)
    bc_d = nc.declare_dram_parameter("bc", [128, 896], bf16, isOutput=False)
    bn_d = nc.declare_dram_parameter("bn", [128, 896], bf16, isOutput=False)
    bn1_d = nc.declare_dram_parameter("bn1", [128, 1920], bf16, isOutput=False)
    btp_d = nc.declare_dram_parameter("btp", [128, 2, 1408], bf16, isOutput=False)
    btm_d = nc.declare_dram_parameter("btm", [128, 2, 1408], bf16, isOutput=False)
    out_d = nc.declare_dram_parameter("out", [B_SHARD, N], bf16, isOutput=True)

    with tile.TileContext(nc) as tc:
        with (
            tc.tile_pool(name="const", bufs=1) as constp,
            tc.tile_pool(name="xb", bufs=3) as xbp,
            tc.tile_pool(name="fold", bufs=2) as fp,
            tc.tile_pool(name="uf", bufs=2) as up,
            tc.tile_pool(name="psum", bufs=1, space="PSUM") as pp,
        ):
            bc = constp.tile([128, 896], bf16)
            bn = constp.tile([128, 896], bf16)
            bn1 = constp.tile([128, 1920], bf16)
            btp = constp.tile([128, 2, 1408], bf16)
            btm = constp.tile([128, 2, 1408], bf16)

            warm_in = constp.tile([128, 512], bf16, name="warm_in")
            nc.vector.memset(warm_in[:], 0.0)

            def emit_dma_in(blk):
                xbig = xbp.tile([128, 32, 128], bf16, tag="xbig", name="xbig")
                nc.sync.dma_start(xbig[:], xt_d[blk])
                return xbig

            def emit_tfolds(xbig, blk):
                xm = fp.tile([128, 16, 128], bf16, tag="xm", name="xm")
                lab(nc.vector.tensor_tensor(
                    xm[:], xbig[:, 16:32, :], xbig[:, 0:16, :], SUB
                ), f"b{blk}.xm")
                e = fp.tile([128, 4, 128], bf16, tag="e", name="e")
                lab(nc.vector.tensor_tensor(
                    e[:], xm[:, 8:12, :], xm[:, 0:4, :], ADD
                ), f"b{blk}.e")
                d = fp.tile([128, 4, 128], bf16, tag="d", name="d")
                lab(nc.vector.tensor_tensor(
                    d[:], xm[:, 12:16, :], xm[:, 4:8, :], SUB
                ), f"b{blk}.d")
                tpfa = fp.tile([128, 4, 128], bf16, tag="tpfa", name="tpfa")
                tpfb = fp.tile([128, 4, 128], bf16, tag="tpfb", name="tpfb")
                tmfa = fp.tile([128, 4, 128], bf16, tag="tmfa", name="tmfa")
                tmfb = fp.tile([128, 4, 128], bf16, tag="tmfb", name="tmfb")
                lab(nc.vector.scalar_tensor_tensor(
                    tpfa[:], e[:], ISQ2, xm[:, 4:8, :], MUL, ADD
                ), f"b{blk}.tpf1")
                lab(nc.vector.scalar_tensor_tensor(
                    tpfb[:], d[:], ISQ2, xm[:, 0:4, :], MUL, SUB
                ), f"b{blk}.tpf2")
                lab(nc.vector.scalar_tensor_tensor(
                    tmfa[:], e[:], ISQ2, xm[:, 4:8, :], MUL, SUB
                ), f"b{blk}.tmf1")
                lab(nc.vector.scalar_tensor_tensor(
                    tmfb[:], d[:], ISQ2, xm[:, 0:4, :], MUL, ADD
                ), f"b{blk}.tmf2")
                return (tpfa, tpfb), (tmfa, tmfb)

            def emit_pfolds(xbig, blk):
                xp = fp.tile([128, 16, 128], bf16, tag="xp", name="xp")
                lab(nc.vector.tensor_tensor(
                    xp[:], xbig[:, 0:16, :], xbig[:, 16:32, :], ADD
                ), f"b{blk}.xpf")
                xpp = fp.tile([128, 8, 128], bf16, tag="xpp", name="xpp")
                lab(nc.vector.tensor_tensor(
                    xpp[:], xp[:, 8:16, :], xp[:, 0:8, :], ADD
                ), f"b{blk}.xpp")
                xpm = fp.tile([128, 8, 128], bf16, tag="xpm", name="xpm")
                lab(nc.vector.tensor_tensor(
                    xpm[:], xp[:, 8:16, :], xp[:, 0:8, :], SUB
                ), f"b{blk}.xpm")
                xc = fp.tile([128, 4, 128], bf16, tag="xc", name="xc")
                lab(nc.gpsimd.tensor_tensor(
                    xc[:], xpp[:, 4:8, :], xpp[:, 0:4, :], ADD
                ), f"b{blk}.xcf")
                xn = fp.tile([128, 4, 128], bf16, tag="xn", name="xn")
                lab(nc.gpsimd.tensor_tensor(
                    xn[:], xpp[:, 4:8, :], xpp[:, 0:4, :], SUB
                ), f"b{blk}.xnf")
                return xpm, xc, xn

            def alloc_psum():
                tAp = pp.tile([128, 1024], f32, tag="tAp", name="tAp")
                tAm = pp.tile([128, 1024], f32, tag="tAm", name="tAm")
                tBn1 = pp.tile([128, 1024], f32, tag="tBn1", name="tBn1")
                tBc = pp.tile([128, 512], f32, tag="tBc", name="tBc")
                tBn = pp.tile([128, 512], f32, tag="tBn", name="tBn")
                return tAp, tAm, tBn1, tBc, tBn

            def mm_t(tAp, tAm, tpf, tmf, blk):
                for part, dst in ((0, "l"), (1, "h")):
                    for kk in range(8):
                        lab(nc.tensor.matmul(
                            tAp[:, 512 * part : 512 * part + 512],
                            tpf[kk // 4][:, kk % 4, :],
                            btp[:, part, 128 * kk : 128 * kk + 512],
                            start=(kk == 0), stop=(kk == 7),
                        ), f"b{blk}.tp{dst}{kk}")
                for part, dst in ((0, "l"), (1, "h")):
                    for kk in range(8):
                        lab(nc.tensor.matmul(
                            tAm[:, 512 * part : 512 * part + 512],
                            tmf[kk // 4][:, kk % 4, :],
                            btm[:, part, 128 * kk : 128 * kk + 512],
                            start=(kk == 0), stop=(kk == 7),
                        ), f"b{blk}.tm{dst}{kk}")

            def mm_cn(tBc, tBn, xc, xn, blk):
                for kk in range(4):
                    lab(nc.tensor.matmul(
                        tBc[:], xc[:, kk, :],
                        bc[:, 128 * kk : 128 * kk + 512],
                        start=(kk == 0), stop=(kk == 3),
                    ), f"b{blk}.c{kk}")
                for kk in range(4):
                    lab(nc.tensor.matmul(
                        tBn[:], xn[:, kk, :],
                        bn[:, 128 * kk : 128 * kk + 512],
                        start=(kk == 0), stop=(kk == 3),
                    ), f"b{blk}.nn{kk}")

            def mm_n1(tBn1, xpm, blk):
                for part, dst in ((0, "l"), (1, "h")):
                    for kk in range(8):
                        lab(nc.tensor.matmul(
                            tBn1[:, 512 * part : 512 * part + 512], xpm[:, kk, :],
                            bn1[:, 128 * kk + 512 * part :
                                 128 * kk + 512 * part + 512],
                            start=(kk == 0), stop=(kk == 7),
                        ), f"b{blk}.n1{dst}{kk}")

            def emit_warm(tAp, n):
                for _ in range(n):
                    nc.tensor.matmul(
                        tAp[:, 0:256], warm_in[:, 0:128], warm_in[:, 0:256],
                        start=True, stop=True,
                    )

            def emit_copies_ct(tAp, tAm, blk=0):
                ctp = up.tile([128, 1024], bf16, tag="ctp", name="ctp")
                ctm = up.tile([128, 1024], bf16, tag="ctm", name="ctm")
                lab(nc.scalar.mul(ctp[:], tAp[:], 1.0), f"b{blk}.Ctp")
                lab(nc.scalar.mul(ctm[:], tAm[:], 1.0), f"b{blk}.Ctm")
                return ctp, ctm

            def emit_copies_cs(tBn1, tBc, tBn, blk=0):
                cn1 = up.tile([128, 1024], bf16, tag="cn1", name="cn1")
                ccn = up.tile([128, 1024], bf16, tag="ccn", name="ccn")
                lab(nc.scalar.mul(cn1[:], tBn1[:], 1.0), f"b{blk}.Cn1")
                lab(nc.scalar.mul(ccn[:, 0:512], tBc[:], 1.0), f"b{blk}.Cc")
                lab(nc.scalar.mul(ccn[:, 512:1024], tBn[:], 1.0), f"b{blk}.Cn")
                return cn1, ccn

            def emit_unfold(blk, ctp, ctm, cn1, ccn):
                p1 = up.tile([128, 1024], bf16, tag="p1", name="p1")
                lab(nc.gpsimd.tensor_tensor(
                    p1[:, 0:512], ccn[:, 0:512], ccn[:, 512:1024], ADD
                ), f"u{blk}.p1l")
                lab(nc.gpsimd.tensor_tensor(
                    p1[:, 512:1024], ccn[:, 0:512], ccn[:, 512:1024], SUB
                ), f"u{blk}.p1h")
                p2 = up.tile([128, 2048], bf16, tag="p2", name="p2")
                lab(nc.gpsimd.tensor_tensor(
                    p2[:, 0:1024], p1[:], cn1[:], ADD
                ), f"u{blk}.p2l")
                lab(nc.gpsimd.tensor_tensor(
                    p2[:, 1024:2048], p1[:], cn1[:], SUB
                ), f"u{blk}.p2h")
                m2 = up.tile([128, 2048], bf16, tag="m2", name="m2")
                lab(nc.vector.tensor_tensor(
                    m2[:, 1536:2048], ctm[:, 0:512], ctp[:, 0:512], SUB
                ), f"u{blk}.m2bh")
                lab(nc.vector.tensor_tensor(
                    m2[:, 1024:1536], ctp[:, 512:1024], ctm[:, 512:1024], SUB
                ), f"u{blk}.m2bl")
                dtet = up.tile([128, 1024], bf16, tag="dtet", name="dtet")
                lab(nc.vector.tensor_tensor(
                    dtet[:], ctp[:], ctm[:], ADD
                ), f"u{blk}.dtet")
                lab(nc.vector.scalar_tensor_tensor(
                    m2[:, 0:512], dtet[:, 0:512], SQ2, m2[:, 1024:1536], MUL, ADD
                ), f"u{blk}.m2al")
                lab(nc.vector.scalar_tensor_tensor(
                    m2[:, 512:1024], dtet[:, 512:1024], SQ2, m2[:, 1536:2048],
                    MUL, SUB
                ), f"u{blk}.m2ah")
                olo = up.tile([128, 2048], bf16, tag="olo", name="olo")
                lab(nc.vector.tensor_tensor(olo[:], p2[:], m2[:], ADD),
                    f"u{blk}.olo")
                nc.sync.dma_start(out_d[128 * blk : 128 * blk + 128, 0:2048], olo[:])
                ohi = up.tile([128, 2048], bf16, tag="ohi", name="ohi")
                lab(nc.vector.tensor_tensor(ohi[:], p2[:], m2[:], SUB),
                    f"u{blk}.ohi")
                nc.sync.dma_start(
                    out_d[128 * blk : 128 * blk + 128, 2048:4096], ohi[:]
                )

            xbigs = {0: emit_dma_in(0)}
            nc.sync.dma_start(btp[:, 0, :], btp_d[:, 0, :])
            nc.sync.dma_start(btp[:, 1, :], btp_d[:, 1, :])
            nc.sync.dma_start(btm[:, 0, :], btm_d[:, 0, :])
            nc.sync.dma_start(btm[:, 1, :], btm_d[:, 1, :])
            nc.sync.dma_start(bn1[:], bn1_d[:])
            nc.sync.dma_start(bc[:], bc_d[:])
            nc.sync.dma_start(bn[:], bn_d[:])
            xbigs[1] = emit_dma_in(1)

            tf = {0: emit_tfolds(xbigs[0], 0)}
            pf = {0: emit_pfolds(xbigs[0], 0)}
            tAp, tAm, tBn1, tBc, tBn = alloc_psum()
            emit_warm(tAp, 18)
            tpf, tmf = tf.pop(0)
            xpm, xc, xn = pf.pop(0)
            mm_t(tAp, tAm, tpf, tmf, 0)
            mm_n1(tBn1, xpm, 0)
            mm_cn(tBc, tBn, xc, xn, 0)
            ct_ = {0: emit_copies_ct(tAp, tAm, 0)}
            xbigs[2] = emit_dma_in(2)
            tf[1] = emit_tfolds(xbigs[1], 1)
            cs_ = {0: emit_copies_cs(tBn1, tBc, tBn, 0)}
            pf[1] = emit_pfolds(xbigs.pop(1), 1)

            for i in range(1, NB - 1):
                tpf, tmf = tf.pop(i)
                xpm, xc, xn = pf.pop(i)
                tAp, tAm, tBn1, tBc, tBn = alloc_psum()
                mm_t(tAp, tAm, tpf, tmf, i)
                mm_n1(tBn1, xpm, i)
                mm_cn(tBc, tBn, xc, xn, i)
                ct_[i] = emit_copies_ct(tAp, tAm, i)
                tf[i + 1] = emit_tfolds(xbigs[i + 1], i + 1)
                cs_[i] = emit_copies_cs(tBn1, tBc, tBn, i)
                emit_unfold(i - 1, *ct_.pop(i - 1), *cs_.pop(i - 1))
                pf[i + 1] = emit_pfolds(xbigs.pop(i + 1), i + 1)
                if i + 2 < NB:
                    xbigs[i + 2] = emit_dma_in(i + 2)

            i = NB - 1
            b0 = 128 * i
            tpf, tmf = tf.pop(i)
            xpm, xc, xn = pf.pop(i)
            emit_unfold(NB - 2, *ct_.pop(NB - 2), *cs_.pop(NB - 2))
            tAp, tAm, tBn1, tBc, tBn = alloc_psum()
            mm_t(tAp, tAm, tpf, tmf, 99)
            ctp, ctm = emit_copies_ct(tAp, tAm, 99)
            mm_cn(tBc, tBn, xc, xn, 99)
            ccn = up.tile([128, 1024], bf16, tag="ccn", name="ccn")
            lab(nc.scalar.mul(ccn[:, 0:512], tBc[:], 1.0), "t.Cc")
            lab(nc.scalar.mul(ccn[:, 512:1024], tBn[:], 1.0), "t.Cn")
            mm_n1(tBn1, xpm, 99)
            m2 = up.tile([128, 2048], bf16, tag="m2", name="m2")
            lab(nc.vector.tensor_tensor(
                m2[:, 1536:2048], ctm[:, 0:512], ctp[:, 0:512], SUB
            ), "t.m2bh")
            lab(nc.vector.tensor_tensor(
                m2[:, 1024:1536], ctp[:, 512:1024], ctm[:, 512:1024], SUB
            ), "t.m2bl")
            dtet = up.tile([128, 1024], bf16, tag="dtet", name="dtet")
            lab(nc.vector.tensor_tensor(dtet[:], ctp[:], ctm[:], ADD), "t.dtet")
            lab(nc.vector.scalar_tensor_tensor(
                m2[:, 0:512], dtet[:, 0:512], SQ2, m2[:, 1024:1536], MUL, ADD
            ), "t.m2al")
            lab(nc.vector.scalar_tensor_tensor(
                m2[:, 512:1024], dtet[:, 512:1024], SQ2, m2[:, 1536:2048], MUL, SUB
            ), "t.m2ah")
            p1 = up.tile([128, 1024], bf16, tag="p1", name="p1")
            lab(nc.vector.tensor_tensor(
                p1[:, 0:512], ccn[:, 0:512], ccn[:, 512:1024], ADD
            ), "t.p1l")
            lab(nc.vector.tensor_tensor(
                p1[:, 512:1024], ccn[:, 0:512], ccn[:, 512:1024], SUB
            ), "t.p1h")
            cn1 = up.tile([128, 1024], bf16, tag="cn1", name="cn1")
            lab(nc.scalar.mul(cn1[:], tBn1[:], 1.0), "t.Cn1")
            p2 = up.tile([128, 2048], bf16, tag="p2", name="p2")
            lab(nc.vector.tensor_tensor(p2[:, 0:1024], p1[:], cn1[:], ADD),
                "t.p2l")
            olo = up.tile([128, 2048], bf16, tag="olo", name="olo")
            ohi = up.tile([128, 2048], bf16, tag="ohi", name="ohi")
            lab(nc.vector.tensor_tensor(
                olo[:, 0:1024], p2[:, 0:1024], m2[:, 0:1024], ADD
            ), "t.olol")
            nc.sync.dma_start(out_d[b0 : b0 + 128, 0:1024], olo[:, 0:1024])
            lab(nc.vector.tensor_tensor(p2[:, 1024:2048], p1[:], cn1[:], SUB),
                "t.p2h")
            lab(nc.vector.tensor_tensor(
                olo[:, 1024:2048], p2[:, 1024:2048], m2[:, 1024:2048], ADD
            ), "t.olor")
            nc.sync.dma_start(out_d[b0 : b0 + 128, 1024:2048], olo[:, 1024:2048])
            lab(nc.vector.tensor_tensor(
                ohi[:, 0:1024], p2[:, 0:1024], m2[:, 0:1024], SUB
            ), "t.ohil")
            nc.sync.dma_start(out_d[b0 : b0 + 128, 2048:3072], ohi[:, 0:1024])
            lab(nc.vector.tensor_tensor(
                ohi[:, 1024:2048], p2[:, 1024:2048], m2[:, 1024:2048], SUB
            ), "t.ohir")
            nc.sync.dma_start(out_d[b0 : b0 + 128, 3072:4096], ohi[:, 1024:2048])

    nc.compile()
    return nc


def _get_nc():
    if "nc" not in _STATE:
        _STATE["nc"] = _build()
    return _STATE["nc"]


def _pad_slice(a, lo, hi):
    out = np.zeros(hi - lo)
    s, e = max(0, lo), min(len(a), hi)
    if e > s:
        out[s - lo : e - lo] = a[s:e]
    return out


def _build_bands(w):
    W0 = np.asarray(w, dtype=np.float64)
    W = np.roll(W0[::-1], 1)
    SQ = np.sqrt(2.0)
    g_c = 1.0 / 8.0
    g_n1 = 1.0 / 4.0
    g_t = 1.0 / (4.0 * SQ)

    Wp2048 = W[:2048] + W[2048:]
    Wm2048 = W[:2048] - W[2048:]
    Wp1024 = Wp2048[:1024] + Wp2048[1024:]
    Wn1024 = Wp2048[:1024] - Wp2048[1024:]
    Wc512 = Wp1024[:512] + Wp1024[512:]
    Wn512 = Wp1024[:512] - Wp1024[512:]

    def tri_reduce(P, alpha):
        mc = len(P) // 2
        A, Bb = P[:mc], P[mc:]
        h = mc // 2
        lo = A[:h] - Bb[:h] - alpha * Bb[h:]
        hi = A[h:] + alpha * Bb[:h] + (alpha * alpha - 1.0) * Bb[h:]
        return np.concatenate([lo, hi])

    def tri_G(WT, m, alpha):
        h = m // 2
        width = m + h - 1
        Glo = (
            _pad_slice(WT, 1 - m, 1 - m + width)
            - _pad_slice(WT, 1, 1 + width)
            - alpha * _pad_slice(WT, 1 + h, 1 + h + width)
        )
        Ghi = (
            _pad_slice(WT, 1 - h, 1 - h + width)
            + alpha * _pad_slice(WT, 1, 1 + width)
            + (alpha * alpha - 1.0) * _pad_slice(WT, 1 + h, 1 + h + width)
        )
        return Glo, Ghi

    def shear(G, ncols):
        Gp = np.zeros(127 + ncols)
        n = min(len(G), 127 + ncols)
        Gp[:n] = G[:n]
        return np.lib.stride_tricks.sliding_window_view(Gp, ncols)[:128].copy()

    def shear2(Glo, Ghi, ncols):
        return np.stack([shear(Glo, ncols), shear(Ghi, ncols)], axis=1)

    t = np.arange(1023)
    Gc = g_c * Wc512[(1 + t) % 512]
    v = t - 511
    Gn = g_c * np.where(v >= 0, 1.0, -1.0) * Wn512[v % 512]
    t1 = np.arange(2047)
    v1 = t1 - 1023
    Gn1 = g_n1 * np.where(v1 >= 0, 1.0, -1.0) * Wn1024[v1 % 1024]
    WTp = tri_reduce(Wm2048, SQ)
    WTm = tri_reduce(Wm2048, -SQ)
    bands = {
        "bc": shear(Gc, 896),
        "bn": shear(Gn, 896),
        "bn1": shear(Gn1, 1920),
        "btp": (g_t * SQ) * shear2(*tri_G(WTp, 1024, SQ), 1408),
        "btm": (g_t * SQ) * shear2(*tri_G(WTm, 1024, -SQ), 1408),
    }
    return {
        k: np.ascontiguousarray(v, dtype=ml_dtypes.bfloat16)
        for k, v in bands.items()
    }


def _prep_inputs(x, w):
    x = np.asarray(x, dtype=np.float32)
    bands = _build_bands(np.asarray(w, dtype=np.float64))
    in_maps = []
    for i in range(N_CORES):
        xs = x[i * B_SHARD : (i + 1) * B_SHARD]
        X = xs[:, ::-1].T
        X4 = X.reshape(32, 128, NB, 128)
        xt = np.ascontiguousarray(
            X4.transpose(2, 1, 0, 3).reshape(NB, 128, 4096),
            dtype=ml_dtypes.bfloat16,
        )
        in_maps.append({"xt": xt, **bands})
    return in_maps


def kernel(x, w, _trace=False):
    from concourse.bass_utils import run_bass_kernel_spmd

    nc = _get_nc()
    in_maps = _prep_inputs(x, w)
    res = run_bass_kernel_spmd(nc, in_maps, list(range(N_CORES)), trace=_trace)
    out = np.concatenate(
        [res.results[i]["out"].astype(np.float32) for i in range(N_CORES)], axis=0
    )
    if _trace:
        _STATE["last_result"] = res
    return out
